# revision 1
# baseline (speedup 1.0000x reference)
"""MoE experts FFN kernel for Trainium2 (8 NeuronCores, expert parallel).

Reference computation (per expert e of 8):
    inter = hidden_states[e] @ gate_up_w[e]        # (C,H)@(H,2I) -> (C,2I)
    gate, up = split(inter, 2, axis=-1)
    act = silu(gate) * up                          # (C,I)
    out[e] = act @ down_w[e]                       # (C,I)@(I,H) -> (C,H)

E == n_cores == 8, so each core owns one expert end-to-end (no collectives).

Device-side layout trick: the PE computes lhsT.T @ rhs with the contraction
dim on partitions for BOTH operands.  Feeding x transposed (Xt = x.T, [H,C])
lets every matmul use naturally-laid-out weights as the stationary operand
and produces transposed intermediates:

    interT[f, c] = sum_h Wgu[h, f] * Xt[h, c]      (lhsT = Wgu tile, rhs = Xt)
    actT          = silu(gateT) * upT              (elementwise, any layout)
    outT[h, c]   = sum_i Wd[i, h] * actT[i, c]     (lhsT = Wd tile, rhs = actT)

The host transposes x on the way in and outT on the way out, casts inputs to
bf16 (fp32 accumulate in PSUM keeps the contraction accurate), and pre-packs
the weights into per-output-block tiles -- [block, p, kt, f] -- so every
weight-group load is one fully-contiguous DMA with 4KB-per-partition runs.
"""

from contextlib import ExitStack

import numpy as np
import ml_dtypes

E, C, H, I = 8, 2048, 2048, 2816
F2 = 2 * I          # fused gate+up columns
P = 128             # partitions
NF = 512            # matmul moving free dim == one PSUM bank of fp32
KT = H // P         # 16 k-tiles over H (matmul 1)
IT = I // P         # 22 i-tiles over I
HT = H // P         # 16 h-tiles over H (matmul 2 output)
FT = F2 // P        # 44 f-blocks (22 gate + 22 up)
CT = C // NF        # 4 c-chunks

_NC_CACHE = {}


def _build_nc(compute="bfloat16"):
    if compute in _NC_CACHE:
        return _NC_CACHE[compute]

    import concourse.bacc as bacc
    import concourse.tile as tile
    from concourse import mybir

    cdt = getattr(mybir.dt, compute)
    f32 = mybir.dt.float32
    AFT = mybir.ActivationFunctionType

    nc = bacc.Bacc(None, target_bir_lowering=False, name="moe_expert_ffn")

    # wgu/wd arrive pre-packed: [block, p, kt* , f] with (kt, f) contiguous
    # per partition p (see make_in_maps).
    xt_d = nc.dram_tensor("xt", [H, C], cdt, kind="ExternalInput")
    wgu_d = nc.dram_tensor("wgu", [FT, P, KT, P], cdt, kind="ExternalInput")
    wd_d = nc.dram_tensor("wd", [HT, P, IT, P], cdt, kind="ExternalInput")
    odt = cdt if compute == "float16" else f32
    outT_d = nc.dram_tensor("outT", [H, C], odt, kind="ExternalOutput")

    xt_r = xt_d.ap().rearrange("(kt p) c -> p kt c", p=P)       # [128, KT, C]
    wgu_a = wgu_d.ap()
    wd_a = wd_d.ap()
    outT_a = outT_d.ap()

    with tile.TileContext(nc) as tc, ExitStack() as ctx:
        singles = ctx.enter_context(tc.tile_pool(name="singles", bufs=1))
        wpool = ctx.enter_context(tc.tile_pool(name="wpool", bufs=2))
        tpool = ctx.enter_context(tc.tile_pool(name="tpool", bufs=2))
        opool = ctx.enter_context(tc.tile_pool(name="opool", bufs=3))
        psum = ctx.enter_context(tc.tile_pool(name="psum", bufs=4, space="PSUM"))

        def glu(i, c, g_ps, u_ps):
            c0 = c * NF
            s_sb = tpool.tile([P, NF], f32, tag="sig", name=f"sig{i}_{c}")
            nc.scalar.activation(out=s_sb, in_=g_ps, func=AFT.Sigmoid)
            nc.vector.tensor_mul(s_sb, g_ps, s_sb)
            nc.vector.tensor_mul(act_sb[i][:, c0 : c0 + NF], s_sb, u_ps)

        def load_gu_weights(i, bufs=3):
            wg = wpool.tile(
                [P, KT, P], cdt, tag="wg", name=f"wg{i}", bufs=bufs
            )
            wu = wpool.tile(
                [P, KT, P], cdt, tag="wu", name=f"wu{i}", bufs=bufs
            )
            nc.sync.dma_start(out=wg, in_=wgu_a[i])
            nc.sync.dma_start(out=wu, in_=wgu_a[IT + i])
            return wg, wu

        # Dummy matmuls on zeroed tiles fill the PE's dead window while the
        # first inputs stream in: HAM un-throttles (1.2 -> 2.4 GHz) after
        # ~3.4us of sustained activity, so the real matmuls start warm
        # instead of paying the cold-clock ramp.  Sized to end just before
        # data arrives (~14us) so they never delay real work, with the gap
        # kept under the ~3.4us re-throttle window.
        wz = singles.tile([P, P], cdt, tag="wz", name="wz")
        xz = singles.tile([P, NF], cdt, tag="xz", name="xz")
        nc.vector.memset(wz, 0.0)
        nc.vector.memset(xz, 0.0)
        warm_ps = psum.tile([P, NF], f32, tag="gps", name="warm_ps")
        for w in range(24):
            nc.tensor.matmul(warm_ps, wz, xz, start=True, stop=True)

        # kt=0 of Xt lands as four small chunk tiles, DMA'd before everything
        # else, so the first matmuls fire during NEFF bring-up instead of
        # behind the whole 8.4MB Xt load.  i=0's weights go next.
        xt0_sb = []
        for c in range(CT):
            t = singles.tile([P, NF], cdt, tag=f"xt0_{c}", name=f"xt0_{c}")
            nc.sync.dma_start(out=t, in_=xt_r[:, 0, c * NF : (c + 1) * NF])
            xt0_sb.append(t)

        wg0, wu0 = load_gu_weights(0)

        # Rest of Xt resident in SBUF, one tile per k-tile so DMAs/readers
        # don't false-share dependency state.
        xt_sb = [None]
        for kt in range(1, KT):
            t = singles.tile([P, C], cdt, tag=f"xt{kt}", name=f"xt{kt}")
            nc.sync.dma_start(out=t, in_=xt_r[:, kt, :])
            xt_sb.append(t)

        def xt_ap(kt, c):
            if kt == 0:
                return xt0_sb[c][:, :]
            return xt_sb[kt][:, c * NF : (c + 1) * NF]

        # actT resident in SBUF, one tile per i-tile.
        act_sb = [
            singles.tile([P, C], cdt, tag=f"act{i}", name=f"act{i}")
            for i in range(IT)
        ]

        # ---- phase 2: interT = Wgu.T @ Xt, actT = silu(gateT)*upT ----
        # i=0 runs kt-outer across all 8 PSUM banks: each matmul depends on
        # one xt k-tile only, so compute overlaps the initial Xt load.
        g0_ps = [
            psum.tile([P, NF], f32, tag="gps", name=f"gps0_{c}") for c in range(CT)
        ]
        u0_ps = [
            psum.tile([P, NF], f32, tag="ups", name=f"ups0_{c}") for c in range(CT)
        ]
        for kt in range(KT):
            for w_t, ps in ((wg0, g0_ps), (wu0, u0_ps)):
                for c in range(CT):
                    c0 = c * NF
                    nc.tensor.matmul(
                        ps[c],
                        w_t[:, kt, :],
                        xt_ap(kt, c),
                        start=(kt == 0),
                        stop=(kt == KT - 1),
                    )
        for c in range(CT):
            glu(0, c, g0_ps[c], u0_ps[c])

        for i in range(1, IT):
            wg, wu = load_gu_weights(i)
            for c in range(CT):
                c0 = c * NF
                g_ps = psum.tile([P, NF], f32, tag="gps", name=f"gps{i}_{c}")
                u_ps = psum.tile([P, NF], f32, tag="ups", name=f"ups{i}_{c}")
                for kt in range(KT):
                    nc.tensor.matmul(
                        g_ps,
                        wg[:, kt, :],
                        xt_ap(kt, c),
                        start=(kt == 0),
                        stop=(kt == KT - 1),
                    )
                for kt in range(KT):
                    nc.tensor.matmul(
                        u_ps,
                        wu[:, kt, :],
                        xt_ap(kt, c),
                        start=(kt == 0),
                        stop=(kt == KT - 1),
                    )
                glu(i, c, g_ps, u_ps)

        # ---- phase 3: outT = Wd.T @ actT ----
        for h in range(HT):
            wd_t = wpool.tile([P, IT, P], cdt, tag="wd", name=f"wd{h}")
            h0 = h * P
            nc.sync.dma_start(out=wd_t, in_=wd_a[h])
            for c in range(CT):
                c0 = c * NF
                # reuse phase-2 bank groups (8 banks total; no room for a
                # third tag)
                o_ps = psum.tile(
                    [P, NF], f32, tag="gps" if c % 2 == 0 else "ups",
                    name=f"ops{h}_{c}",
                )
                for it in range(IT):
                    nc.tensor.matmul(
                        o_ps,
                        wd_t[:, it, :],
                        act_sb[it][:, c0 : c0 + NF],
                        start=(it == 0),
                        stop=(it == IT - 1),
                    )
                o_sb = opool.tile([P, NF], odt, tag="osb", name=f"osb{h}_{c}")
                nc.vector.tensor_copy(out=o_sb, in_=o_ps)
                nc.sync.dma_start(
                    out=outT_a[h0 : h0 + P, c0 : c0 + NF], in_=o_sb
                )

    nc.compile()
    _NC_CACHE[compute] = nc
    return nc


def _np_dtype(compute):
    return {"bfloat16": ml_dtypes.bfloat16, "float16": np.float16, "float32r": np.float32}[compute]


def _pack_w(w, n_k, n_b):
    """[K, B*P] -> [B, P, n_k, P] with (kt, f) contiguous per partition p."""
    return np.ascontiguousarray(
        w.reshape(n_k, P, n_b, P).transpose(2, 1, 0, 3)
    )


def make_in_maps(hidden_states, gate_up_w, down_w, compute="bfloat16"):
    dt = _np_dtype(compute)
    in_maps = []
    for e in range(E):
        in_maps.append(
            {
                "xt": np.ascontiguousarray(hidden_states[e].T).astype(dt),
                "wgu": _pack_w(gate_up_w[e].astype(dt), KT, FT),
                "wd": _pack_w(down_w[e].astype(dt), IT, HT),
            }
        )
    return in_maps


def run_hw(in_maps, compute="bfloat16", trace=False, **kwargs):
    from concourse import bass_utils

    if trace:
        # local-only devloop: skip the artifact-bucket upload
        bass_utils.upload_artifacts = lambda tmpdir: f"local:{tmpdir}"
    nc = _build_nc(compute)
    return bass_utils.run_bass_kernel_spmd(
        nc, in_maps, core_ids=list(range(E)), trace=trace, **kwargs
    )


def kernel(hidden_states, gate_up_w, down_w):
    compute = "float16"
    hidden_states = np.asarray(hidden_states)
    gate_up_w = np.asarray(gate_up_w)
    down_w = np.asarray(down_w)
    in_maps = make_in_maps(hidden_states, gate_up_w, down_w, compute)
    res = run_hw(in_maps, compute)
    out = np.empty((E, C, H), dtype=np.float32)
    for e in range(E):
        out[e] = res.results[e]["outT"].T
    return out



# revision 5
# speedup vs baseline: 1.0196x; 1.0196x over previous
"""MoE experts FFN kernel for Trainium2 (8 NeuronCores, expert parallel).

Reference computation (per expert e of 8):
    inter = hidden_states[e] @ gate_up_w[e]        # (C,H)@(H,2I) -> (C,2I)
    gate, up = split(inter, 2, axis=-1)
    act = silu(gate) * up                          # (C,I)
    out[e] = act @ down_w[e]                       # (C,I)@(I,H) -> (C,H)

E == n_cores == 8, so each core owns one expert end-to-end (no collectives).

Device layout: everything transposed so the PE's contraction dim sits on
partitions for both operands (lhsT = weights stationary, rhs = Xt moving):

    interT[f, c] = sum_h Wgu[h, f] * Xt[h, c]
    outT[h, c]   = sum_i Wd[i, h] * actT[i, c]

The baseline streams 4224 N=512 matmuls at the PE's 1-column/cycle floor
(96.5% MFU) -- the only way faster is fewer matmuls.  mm1 (2/3 of the MACs)
uses one level of Strassen over (F2, H, C):

    A' = WguT in 2x2 blocks over (F2/2=2816, H/2=1024); the F2 split lands
    exactly on the gate|up boundary.  B = Xt in 2x2 blocks over (H, C).

    M1 = (A11+A22)(B11+B22)   M2 = (A21+A22)B11    M3 = A11(B12-B22)
    M4 = A22(B21-B11)         M5 = (A11+A12)B22    M6 = (A21-A11)(B11+B12)
    M7 = (A12-A22)(B21+B22)

    gateT[:, c<1024] = M1+M4-M5+M7      gateT[:, c>=1024] = M3+M5
    upT[:, c<1024]   = M2+M4            upT[:, c>=1024]   = M1-M2+M3+M6

7/8 of the matmuls: 22 f-blocks x 7 products x 2 c-chunks x 8 k-tiles.
The 7 Xt-combos (16KB/partition each) stay resident; weight combos form
per f-block on GpSimd; products drain from PSUM into fp16 accumulators on
the Vector engine (fused +/- via scalar_tensor_tensor); GLU fuses into the
drain.  act doesn't fit next to the combos, so it spills to a DRAM scratch
tile and reloads in 512-column chunks for a c-outer classical mm2.
"""

from contextlib import ExitStack

import numpy as np
import ml_dtypes

E, C, H, I = 8, 2048, 2048, 2816
F2 = 2 * I          # fused gate+up columns
P = 128             # partitions
NF = 512            # matmul moving free dim == one PSUM bank of fp32
KT2 = (H // 2) // P  # 8 k-tiles per H-half (Strassen mm1)
FB = I // P         # 22 f-blocks (gate tile i pairs with up tile i)
IT = I // P         # 22 i-tiles over I (mm2 contraction)
HT = H // P         # 16 h-tiles over H (mm2 output)
CT = C // NF        # 4 c-chunks of 512
CH = C // 2         # 1024, Strassen c-half

_NC_CACHE = {}


def _build_nc(compute="float16"):
    if compute in _NC_CACHE:
        return _NC_CACHE[compute]

    import concourse.bacc as bacc
    import concourse.tile as tile
    from concourse import mybir

    cdt = getattr(mybir.dt, compute)
    f32 = mybir.dt.float32
    AFT = mybir.ActivationFunctionType
    ALU = mybir.AluOpType

    nc = bacc.Bacc(None, target_bir_lowering=False, name="moe_expert_ffn")

    xt_d = nc.dram_tensor("xt", [H, C], cdt, kind="ExternalInput")
    # wgu pre-packed per f-block: [fb, p, 4, kt, f] with quadrant order
    # (G_lo, G_hi, U_lo, U_hi); (quad, kt, f) contiguous per partition p.
    wgu_d = nc.dram_tensor("wgu", [FB, P, 4, KT2, P], cdt, kind="ExternalInput")
    wd_d = nc.dram_tensor("wd", [HT, P, IT, P], cdt, kind="ExternalInput")
    odt = cdt if compute == "float16" else f32
    outT_d = nc.dram_tensor("outT", [H, C], odt, kind="ExternalOutput")

    xt_r = xt_d.ap().rearrange("(kt p) c -> p kt c", p=P)       # [128, 16, C]
    wgu_a = wgu_d.ap()
    wd_a = wd_d.ap()
    outT_a = outT_d.ap()

    with tile.TileContext(nc) as tc, ExitStack() as ctx:
        singles = ctx.enter_context(tc.tile_pool(name="singles", bufs=1))
        wpool = ctx.enter_context(tc.tile_pool(name="wpool", bufs=2))
        accp = ctx.enter_context(tc.tile_pool(name="accp", bufs=2))
        spool = ctx.enter_context(tc.tile_pool(name="spool", bufs=2))
        rpool = ctx.enter_context(tc.tile_pool(name="rpool", bufs=2))
        opool = ctx.enter_context(tc.tile_pool(name="opool", bufs=3))
        psum = ctx.enter_context(tc.tile_pool(name="psum", bufs=4, space="PSUM"))
        dram = ctx.enter_context(tc.tile_pool(name="dram", bufs=1, space="DRAM"))

        # DRAM scratch for the act spill (actT, one tile per i-block).
        act_d = dram.tile([FB, P, C], cdt, tag="actd", name="act_spill")

        # Dummy matmuls on zeroed tiles fill the PE's dead window while the
        # first inputs stream in (HAM un-throttles after ~3.4us of activity).
        wz = singles.tile([P, P], cdt, tag="wz", name="wz")
        xz = singles.tile([P, NF], cdt, tag="xz", name="xz")
        nc.vector.memset(wz, 0.0)
        nc.vector.memset(xz, 0.0)
        warm_ps = psum.tile([P, NF], f32, tag="pa", name="warm_ps")
        for w in range(24):
            nc.tensor.matmul(warm_ps, wz, xz, start=True, stop=True)

        # ---- Xt combos: B1..B7, [P, kt, 1024] each, one tile per (j, kt)
        # slice to avoid false-sharing between the 4 quadrant DMAs, the
        # combo adds, and the PE readers.
        bc = [[None] * KT2 for _ in range(7)]
        for j in range(7):
            for kt in range(KT2):
                bc[j][kt] = singles.tile(
                    [P, CH], cdt, tag=f"b{j}_{kt}", name=f"b{j}_{kt}"
                )
        for kt in range(KT2):
            # raw quadrant slices of Xt for this k-tile
            nc.sync.dma_start(out=bc[1][kt], in_=xt_r[:, kt, 0:CH])        # B11
            nc.sync.dma_start(out=bc[4][kt], in_=xt_r[:, KT2 + kt, CH:C])  # B22
            t12 = spool.tile([P, CH], cdt, tag="q12", name=f"q12_{kt}")
            t21 = spool.tile([P, CH], cdt, tag="q21", name=f"q21_{kt}")
            nc.sync.dma_start(out=t12, in_=xt_r[:, kt, CH:C])              # B12
            nc.sync.dma_start(out=t21, in_=xt_r[:, KT2 + kt, 0:CH])        # B21
            nc.vector.tensor_add(bc[0][kt], bc[1][kt], bc[4][kt])  # B11+B22
            nc.vector.tensor_sub(bc[2][kt], t12, bc[4][kt])        # B12-B22
            nc.vector.tensor_sub(bc[3][kt], t21, bc[1][kt])        # B21-B11
            nc.vector.tensor_add(bc[5][kt], bc[1][kt], t12)        # B11+B12
            nc.vector.tensor_add(bc[6][kt], t21, bc[4][kt])        # B21+B22

        def glu(act_t, chunk, g_sb, u_sb):
            s_sb = spool.tile([P, NF], f32, tag="sig", name=f"sig{chunk}")
            nc.scalar.activation(out=s_sb, in_=g_sb, func=AFT.Sigmoid)
            nc.vector.tensor_mul(s_sb, g_sb, s_sb)
            nc.vector.tensor_mul(act_t[:, chunk * NF : (chunk + 1) * NF], s_sb, u_sb)

        # ---- mm1: per f-block, 7 Strassen products + recombine + GLU ----
        for i in range(FB):
            raw = wpool.tile([P, 4, KT2, P], cdt, tag="wraw", name=f"wraw{i}")
            nc.sync.dma_start(out=raw, in_=wgu_a[i])
            # quadrant APs: 0=G_lo, 1=G_hi, 2=U_lo, 3=U_hi
            quad = [raw[:, q, :, :] for q in range(4)]
            # stationary combos (W3 = G_lo, W4 = U_hi used raw)
            wcs = {}
            for nm, a, b, op in (
                ("w1", quad[0], quad[3], "add"),
                ("w2", quad[2], quad[3], "add"),
                ("w5", quad[0], quad[1], "add"),
                ("w6", quad[2], quad[0], "sub"),
                ("w7", quad[1], quad[3], "sub"),
            ):
                t = wpool.tile([P, KT2, P], cdt, tag=nm, name=f"{nm}_{i}")
                getattr(nc.gpsimd, f"tensor_{op}")(t, a, b)
                wcs[nm] = t
            # per-product stationary operand: (tile/None-quad, quad index)
            wspecs = [(wcs["w1"], None), (wcs["w2"], None), (raw, 0),
                      (raw, 3), (wcs["w5"], None), (wcs["w6"], None),
                      (wcs["w7"], None)]

            # fp16 accumulators: [gate|up] x [c-half] x [chunk]
            acc = {}
            for nm in ("gl", "gh", "ul", "uh"):
                for j in range(2):
                    acc[nm, j] = accp.tile(
                        [P, NF], cdt, tag=f"{nm}{j}", name=f"{nm}{j}_{i}"
                    )

            # per-product drain plan: (acc key, how) where how is
            # "copy" (first write), "add", or "sub" (acc - M via fused op)
            plan = [
                [("gl", "copy"), ("uh", "copy")],          # M1
                [("ul", "copy"), ("uh", "sub")],           # M2
                [("gh", "copy"), ("uh", "add")],           # M3
                [("gl", "add"), ("ul", "add")],            # M4
                [("gh", "add"), ("gl", "sub")],            # M5
                [("uh", "add")],                           # M6
                [("gl", "add")],                           # M7
            ]
            for m in range(7):
                w_t, w_q = wspecs[m]
                for j in range(2):
                    ps = psum.tile(
                        [P, NF], f32, tag="pa" if (m * 2 + j) % 2 == 0 else "pb",
                        name=f"m{m}_{j}_{i}",
                    )
                    for kt in range(KT2):
                        lhsT = (
                            w_t[:, kt, :] if w_q is None
                            else w_t[:, w_q, kt, :]
                        )
                        nc.tensor.matmul(
                            ps,
                            lhsT,
                            bc[m][kt][:, j * NF : (j + 1) * NF],
                            start=(kt == 0),
                            stop=(kt == KT2 - 1),
                        )
                    for key, how in plan[m]:
                        a = acc[key, j]
                        if how == "copy":
                            nc.vector.tensor_copy(out=a, in_=ps)
                        elif how == "add":
                            nc.vector.tensor_add(a, ps, a)
                        else:  # a = (ps * -1) + a
                            nc.vector.scalar_tensor_tensor(
                                out=a, in0=ps, scalar=-1.0, in1=a,
                                op0=ALU.mult, op1=ALU.add,
                            )

            act_t = spool.tile([P, C], cdt, tag="act", name=f"act{i}")
            glu(act_t, 0, acc["gl", 0], acc["ul", 0])
            glu(act_t, 1, acc["gl", 1], acc["ul", 1])
            glu(act_t, 2, acc["gh", 0], acc["uh", 0])
            glu(act_t, 3, acc["gh", 1], acc["uh", 1])
            nc.sync.dma_start(out=act_d[i], in_=act_t)

        # ---- mm2: outT = Wd.T @ actT, c-chunk outer with act reloaded ----
        # Reload tiles reuse the Xt-combo tag buffers (the combos' last
        # readers are mm1's final matmuls), so no extra SBUF is held: tag
        # (cc%2)*22+it serves chunks cc and cc+2.
        for cc in range(CT):
            c0 = cc * NF
            rts = []
            for it in range(IT):
                t_idx = (cc % 2) * 22 + it
                rt = singles.tile(
                    [P, NF], cdt, tag=f"b{t_idx // KT2}_{t_idx % KT2}",
                    name=f"r{it}_{cc}",
                )
                nc.sync.dma_start(out=rt, in_=act_d[it, :, c0 : c0 + NF])
                rts.append(rt)
            for h in range(HT):
                wd_t = rpool.tile([P, IT, P], cdt, tag="wd", name=f"wd{h}_{cc}")
                nc.sync.dma_start(out=wd_t, in_=wd_a[h])
                o_ps = psum.tile(
                    [P, NF], f32, tag="pa" if h % 2 == 0 else "pb",
                    name=f"ops{h}_{cc}",
                )
                for it in range(IT):
                    nc.tensor.matmul(
                        o_ps,
                        wd_t[:, it, :],
                        rts[it],
                        start=(it == 0),
                        stop=(it == IT - 1),
                    )
                o_sb = opool.tile([P, NF], odt, tag="osb", name=f"osb{h}_{cc}")
                nc.vector.tensor_copy(out=o_sb, in_=o_ps)
                h0 = h * P
                nc.sync.dma_start(
                    out=outT_a[h0 : h0 + P, c0 : c0 + NF], in_=o_sb
                )

    nc.compile()
    _NC_CACHE[compute] = nc
    return nc


def _np_dtype(compute):
    return {"bfloat16": ml_dtypes.bfloat16, "float16": np.float16, "float32r": np.float32}[compute]


def _pack_w(w, n_k, n_b):
    """[K, B*P] -> [B, P, n_k, P] with (kt, f) contiguous per partition p."""
    return np.ascontiguousarray(
        w.reshape(n_k, P, n_b, P).transpose(2, 1, 0, 3)
    )


def _pack_wgu_strassen(wgu):
    """[H, F2] -> [FB, P, 4, KT2, P], quadrants (G_lo, G_hi, U_lo, U_hi).

    Quadrant q of f-block i is Wgu[h-half, col-block] reshaped so the k-tile
    index kt and the 128 output columns are contiguous per partition.
    """
    out = np.empty((FB, P, 4, KT2, P), dtype=wgu.dtype)
    for i in range(FB):
        g = wgu[:, i * P : (i + 1) * P]            # [H, 128] gate cols
        u = wgu[:, I + i * P : I + (i + 1) * P]    # [H, 128] up cols
        for q, blk in enumerate((g[: H // 2], g[H // 2 :], u[: H // 2], u[H // 2 :])):
            # [1024, 128] -> [kt, p, f] -> [p, kt, f]
            out[i, :, q] = blk.reshape(KT2, P, P).transpose(1, 0, 2)
    return out


def make_in_maps(hidden_states, gate_up_w, down_w, compute="float16"):
    dt = _np_dtype(compute)
    in_maps = []
    for e in range(E):
        in_maps.append(
            {
                "xt": np.ascontiguousarray(hidden_states[e].T).astype(dt),
                "wgu": _pack_wgu_strassen(gate_up_w[e].astype(dt)),
                "wd": _pack_w(down_w[e].astype(dt), IT, HT),
            }
        )
    return in_maps


def run_hw(in_maps, compute="float16", trace=False, **kwargs):
    from concourse import bass_utils

    if trace:
        # local-only devloop: skip the artifact-bucket upload
        bass_utils.upload_artifacts = lambda tmpdir: f"local:{tmpdir}"
    nc = _build_nc(compute)
    return bass_utils.run_bass_kernel_spmd(
        nc, in_maps, core_ids=list(range(E)), trace=trace, **kwargs
    )


def kernel(hidden_states, gate_up_w, down_w):
    compute = "float16"
    hidden_states = np.asarray(hidden_states)
    gate_up_w = np.asarray(gate_up_w)
    down_w = np.asarray(down_w)
    in_maps = make_in_maps(hidden_states, gate_up_w, down_w, compute)
    res = run_hw(in_maps, compute)
    out = np.empty((E, C, H), dtype=np.float32)
    for e in range(E):
        out[e] = res.results[e]["outT"].T
    return out


# revision 13
# speedup vs baseline: 1.0506x; 1.0304x over previous
"""MoE experts FFN kernel for Trainium2 (8 NeuronCores, expert parallel).

Reference computation (per expert e of 8):
    inter = hidden_states[e] @ gate_up_w[e]        # (C,H)@(H,2I) -> (C,2I)
    gate, up = split(inter, 2, axis=-1)
    act = silu(gate) * up                          # (C,I)
    out[e] = act @ down_w[e]                       # (C,I)@(I,H) -> (C,H)

E == n_cores == 8, so each core owns one expert end-to-end (no collectives).

Device layout: everything transposed so the PE's contraction dim sits on
partitions for both operands (lhsT = weights stationary, rhs = Xt moving):

    interT[f, c] = sum_h Wgu[h, f] * Xt[h, c]
    outT[h, c]   = sum_i Wd[i, h] * actT[i, c]

The classical kernel streams 4224 N=512 matmuls at the PE's 1-column/cycle
floor (96.5% MFU) -- the only way faster is fewer matmuls.  mm1 (2/3 of the
MACs) uses one level of Strassen over (F2, H, C); the F2 split lands exactly
on the gate|up boundary:

    M1 = (A11+A22)(B11+B22)   M2 = (A21+A22)B11    M3 = A11(B12-B22)
    M4 = A22(B21-B11)         M5 = (A11+A12)B22    M6 = (A21-A11)(B11+B12)
    M7 = (A12-A22)(B21+B22)

    gateT[:, c<1024] = M1+M4-M5+M7      gateT[:, c>=1024] = M3+M5
    upT[:, c<1024]   = M2+M4            upT[:, c>=1024]   = M1-M2+M3+M6

7/8 of the matmuls: 22 f-blocks x 7 products x (16 k-tile MMs across two
PSUM banks).  Both operand combos are PRECOMPUTED ON HOST and DMA'd in
(they are pure functions of the inputs), so the device does only matmuls,
PSUM drains (fused +/- on the Vector engine), Silu-GLU, and DMA.  act
spills to DRAM scratch (SBUF holds the 7 Xt-combos instead) and reloads in
512-column chunks for a c-outer classical mm2.
"""

from contextlib import ExitStack

import numpy as np
import ml_dtypes

E, C, H, I = 8, 2048, 2048, 2816
F2 = 2 * I          # fused gate+up columns
P = 128             # partitions
NF = 512            # PSUM bank of fp32; mm chains write [P, 2*NF] tiles
KT2 = (H // 2) // P  # 8 k-tiles per H-half (Strassen mm1)
FB = I // P         # 22 f-blocks (gate tile i pairs with up tile i)
IT = I // P         # 22 i-tiles over I (mm2 contraction)
HT = H // P         # 16 h-tiles over H (mm2 output)
CT = C // NF        # 4 c-chunks of 512
CH = C // 2         # 1024, Strassen c-half

_NC_CACHE = {}


def _build_nc(compute="float16"):
    if compute in _NC_CACHE:
        return _NC_CACHE[compute]

    import concourse.bacc as bacc
    import concourse.tile as tile
    from concourse import mybir

    cdt = getattr(mybir.dt, compute)
    f32 = mybir.dt.float32
    AFT = mybir.ActivationFunctionType
    ALU = mybir.AluOpType

    nc = bacc.Bacc(None, target_bir_lowering=False, name="moe_expert_ffn")

    # xt raw (combos formed on DVE: startup is DMA-bound, so ship the
    # smaller raw tensor); weight combos precomputed on host.
    xt_d = nc.dram_tensor("xt", [H, C], cdt, kind="ExternalInput")
    wc_d = nc.dram_tensor("wc", [FB, P, 7, KT2, P], cdt, kind="ExternalInput")
    wd_d = nc.dram_tensor("wd", [HT, P, IT, P], cdt, kind="ExternalInput")
    odt = cdt if compute == "float16" else f32
    outT_d = nc.dram_tensor("outT", [H, C], odt, kind="ExternalOutput")

    xt_r = xt_d.ap().rearrange("(kt p) c -> p kt c", p=P)       # [128, 16, C]
    wc_a = wc_d.ap()
    wd_a = wd_d.ap()
    outT_a = outT_d.ap()

    with tile.TileContext(nc) as tc, ExitStack() as ctx:
        singles = ctx.enter_context(tc.tile_pool(name="singles", bufs=1))
        wpool = ctx.enter_context(tc.tile_pool(name="wpool", bufs=2))
        accp = ctx.enter_context(tc.tile_pool(name="accp", bufs=2))
        spool = ctx.enter_context(tc.tile_pool(name="spool", bufs=2))
        rpool = ctx.enter_context(tc.tile_pool(name="rpool", bufs=2))
        opool = ctx.enter_context(tc.tile_pool(name="opool", bufs=3))
        psum = ctx.enter_context(tc.tile_pool(name="psum", bufs=2, space="PSUM"))
        dram = ctx.enter_context(tc.tile_pool(name="dram", bufs=1, space="DRAM"))

        # DRAM scratch for the act spill (actT, one tile per i-block).
        act_d = dram.tile([FB, P, C], cdt, tag="actd", name="act_spill")

        # Dummy matmuls on zeroed tiles fill the PE's dead window while the
        # first inputs stream in (HAM un-throttles after ~3.4us of activity).
        wz = singles.tile([P, P], cdt, tag="wz", name="wz")
        xz = singles.tile([P, NF], cdt, tag="xz", name="xz")
        nc.vector.memset(wz, 0.0)
        nc.vector.memset(xz, 0.0)
        warm_ps = psum.tile([P, 2 * NF], f32, tag="pa", name="warm_ps")
        for w in range(24):
            nc.tensor.matmul(
                warm_ps[:, :NF], wz, xz, start=True, stop=True
            )

        # first weight-combo block: highest DMA priority
        wc0 = wpool.tile([P, 7, KT2, P], cdt, tag="wc", name="wc0")
        nc.sync.dma_start(out=wc0, in_=wc_a[0])

        # Xt combos formed per k-tile on the Vector engine as the raw
        # quadrant slices stream in; one tile per (product, kt) slice so
        # DMAs and readers don't false-share dependency state.
        bc = [[None] * KT2 for _ in range(7)]
        for m in range(7):
            for kt in range(KT2):
                bc[m][kt] = singles.tile(
                    [P, CH], cdt, tag=f"b{m}_{kt}", name=f"b{m}_{kt}"
                )
        for kt in range(KT2):
            nc.sync.dma_start(out=bc[1][kt], in_=xt_r[:, kt, 0:CH])        # B11
            nc.sync.dma_start(out=bc[4][kt], in_=xt_r[:, KT2 + kt, CH:C])  # B22
            t12 = spool.tile([P, CH], cdt, tag="q12", name=f"q12_{kt}")
            t21 = spool.tile([P, CH], cdt, tag="q21", name=f"q21_{kt}")
            nc.sync.dma_start(out=t12, in_=xt_r[:, kt, CH:C])              # B12
            nc.sync.dma_start(out=t21, in_=xt_r[:, KT2 + kt, 0:CH])        # B21
            nc.vector.tensor_add(bc[0][kt], bc[1][kt], bc[4][kt])  # B11+B22
            nc.vector.tensor_sub(bc[2][kt], t12, bc[4][kt])        # B12-B22
            nc.vector.tensor_sub(bc[3][kt], t21, bc[1][kt])        # B21-B11
            nc.vector.tensor_add(bc[5][kt], bc[1][kt], t12)        # B11+B12
            nc.vector.tensor_add(bc[6][kt], t21, bc[4][kt])        # B21+B22

        # per-product drain plan: (acc key, how); "sub" is acc - M fused
        plan = [
            [("gl", "copy"), ("uh", "copy")],          # M1
            [("ul", "copy"), ("uh", "sub")],           # M2
            [("gh", "copy"), ("uh", "add")],           # M3
            [("gl", "add"), ("ul", "add")],            # M4
            [("gh", "add"), ("gl", "sub")],            # M5
            [("uh", "add")],                           # M6
            [("gl", "add")],                           # M7
        ]

        # ---- mm1: per f-block, 7 Strassen products + drains + GLU ----
        for i in range(FB):
            wc_t = wc0 if i == 0 else wpool.tile(
                [P, 7, KT2, P], cdt, tag="wc", name=f"wc{i}"
            )
            if i > 0:
                nc.sync.dma_start(out=wc_t, in_=wc_a[i])

            acc = {}
            for nm in ("gl", "gh", "ul", "uh"):
                acc[nm] = accp.tile([P, 2 * NF], cdt, tag=nm, name=f"{nm}_{i}")

            def drain(m, ps):
                for key, how in plan[m]:
                    a = acc[key]
                    if how == "copy":
                        nc.vector.tensor_copy(out=a, in_=ps)
                    elif how == "add":
                        nc.vector.tensor_add(a, ps, a)
                    else:  # a = (ps * -1) + a
                        nc.vector.scalar_tensor_tensor(
                            out=a, in0=ps, scalar=-1.0, in1=a,
                            op0=ALU.mult, op1=ALU.add,
                        )

            def mm(ps, m, kt, j):
                nc.tensor.matmul(
                    ps[:, j * NF : (j + 1) * NF],
                    wc_t[:, m, kt, :],
                    bc[m][kt][:, j * NF : (j + 1) * NF],
                    start=(kt == 0),
                    stop=(kt == KT2 - 1),
                )

            if i == 0:
                # kt-major waves: the PE starts the moment the first k-tile
                # combos exist instead of waiting for full products' worth
                # of Xt (startup is DMA-bound).
                pss = {}
                for wave in ((0, 1, 2, 3), (4, 5, 6)):
                    for m in wave:
                        pss[m] = psum.tile(
                            [P, 2 * NF], f32, tag="pa" if m % 2 == 0 else "pb",
                            name=f"m{m}_{i}",
                        )
                    for kt in range(KT2):
                        for m in wave:
                            for j in range(2):
                                mm(pss[m], m, kt, j)
                for m in range(7):
                    drain(m, pss[m])
            else:
                for m in range(7):
                    ps = psum.tile(
                        [P, 2 * NF], f32, tag="pa" if m % 2 == 0 else "pb",
                        name=f"m{m}_{i}",
                    )
                    for j in range(2):
                        for kt in range(KT2):
                            mm(ps, m, kt, j)
                    drain(m, ps)

            act_t = spool.tile([P, C], cdt, tag="act", name=f"act{i}")
            for half, g, u in ((0, "gl", "ul"), (1, "gh", "uh")):
                s_sb = spool.tile([P, 2 * NF], cdt, tag=f"sig{half}", name=f"sig{half}_{i}")
                nc.scalar.activation(out=s_sb, in_=acc[g], func=AFT.Silu)
                nc.vector.tensor_mul(
                    act_t[:, half * CH : (half + 1) * CH], s_sb, acc[u]
                )
            nc.sync.dma_start(out=act_d[i], in_=act_t)

        # mm2 wd prefetch: emitted after mm1's DMAs (so it never delays
        # them in a shared queue) but dependency-free, so the transfers
        # fill DMA idle time during late mm1.
        wd_tiles = {}
        for h in range(3):
            wd_tiles[h] = rpool.tile(
                [P, IT, P], cdt, tag="wd", name=f"wd{h}", bufs=3
            )
            nc.sync.dma_start(out=wd_tiles[h], in_=wd_a[h])

        # ---- mm2: outT = Wd.T @ actT, c-chunk outer with act reloaded ----
        # Reload tiles reuse the Xt-combo tag buffers (freed as mm1's last
        # f-block consumes them): tag (cc%2)*22+it serves chunks cc, cc+2.
        for cc in range(CT):
            c0 = cc * NF
            rts = []
            for it in range(IT):
                t_idx = (cc % 2) * 22 + it
                rt = singles.tile(
                    [P, NF], cdt, tag=f"b{t_idx // KT2}_{t_idx % KT2}",
                    name=f"r{it}_{cc}",
                )
                nc.sync.dma_start(out=rt, in_=act_d[it, :, c0 : c0 + NF])
                rts.append(rt)
            for h in range(HT):
                if cc == 0 and h < 3:
                    wd_t = wd_tiles[h]
                else:
                    wd_t = rpool.tile(
                        [P, IT, P], cdt, tag="wd", name=f"wd{h}_{cc}", bufs=3
                    )
                    nc.sync.dma_start(out=wd_t, in_=wd_a[h])
                o_ps = psum.tile(
                    [P, 2 * NF], f32, tag="pa" if h % 2 == 0 else "pb",
                    name=f"ops{h}_{cc}",
                )
                for it in range(IT):
                    nc.tensor.matmul(
                        o_ps[:, :NF],
                        wd_t[:, it, :],
                        rts[it],
                        start=(it == 0),
                        stop=(it == IT - 1),
                    )
                o_sb = opool.tile([P, NF], odt, tag="osb", name=f"osb{h}_{cc}")
                nc.scalar.activation(out=o_sb, in_=o_ps[:, :NF], func=AFT.Copy)
                h0 = h * P
                nc.sync.dma_start(
                    out=outT_a[h0 : h0 + P, c0 : c0 + NF], in_=o_sb
                )

    nc.compile()
    _NC_CACHE[compute] = nc
    return nc


def _np_dtype(compute):
    return {"bfloat16": ml_dtypes.bfloat16, "float16": np.float16, "float32r": np.float32}[compute]


def _pack_w(w, n_k, n_b):
    """[K, B*P] -> [B, P, n_k, P] with (kt, f) contiguous per partition p."""
    return np.ascontiguousarray(
        w.reshape(n_k, P, n_b, P).transpose(2, 1, 0, 3)
    )


def _pack_wc(wgu):
    """[H, F2] -> Strassen stationary combos [FB, P, 7, KT2, P]."""
    out = np.empty((FB, P, 7, KT2, P), dtype=wgu.dtype)
    for i in range(FB):
        g = wgu[:, i * P : (i + 1) * P]            # [H, 128] gate cols
        u = wgu[:, I + i * P : I + (i + 1) * P]    # [H, 128] up cols
        g_lo, g_hi = g[: H // 2], g[H // 2 :]
        u_lo, u_hi = u[: H // 2], u[H // 2 :]
        combos = (g_lo + u_hi, u_lo + u_hi, g_lo, u_hi, g_lo + g_hi,
                  u_lo - g_lo, g_hi - u_hi)
        for m, cb in enumerate(combos):
            # [1024, 128] -> [kt, p, f] -> [p, kt, f]
            out[i, :, m] = cb.reshape(KT2, P, P).transpose(1, 0, 2)
    return out


def make_in_maps(hidden_states, gate_up_w, down_w, compute="float16"):
    dt = _np_dtype(compute)
    in_maps = []
    for e in range(E):
        in_maps.append(
            {
                "xt": np.ascontiguousarray(hidden_states[e].T).astype(dt),
                "wc": _pack_wc(gate_up_w[e].astype(dt)),
                "wd": _pack_w(down_w[e].astype(dt), IT, HT),
            }
        )
    return in_maps


def run_hw(in_maps, compute="float16", trace=False, **kwargs):
    from concourse import bass_utils

    if trace:
        # local-only devloop: skip the artifact-bucket upload
        bass_utils.upload_artifacts = lambda tmpdir: f"local:{tmpdir}"
    nc = _build_nc(compute)
    return bass_utils.run_bass_kernel_spmd(
        nc, in_maps, core_ids=list(range(E)), trace=trace, **kwargs
    )


def kernel(hidden_states, gate_up_w, down_w):
    compute = "float16"
    hidden_states = np.asarray(hidden_states)
    gate_up_w = np.asarray(gate_up_w)
    down_w = np.asarray(down_w)
    in_maps = make_in_maps(hidden_states, gate_up_w, down_w, compute)
    res = run_hw(in_maps, compute)
    out = np.empty((E, C, H), dtype=np.float32)
    for e in range(E):
        out[e] = res.results[e]["outT"].T
    return out


# revision 20
# speedup vs baseline: 1.0617x; 1.0106x over previous
"""MoE experts FFN kernel for Trainium2 (8 NeuronCores, expert parallel).

Reference computation (per expert e of 8):
    inter = hidden_states[e] @ gate_up_w[e]        # (C,H)@(H,2I) -> (C,2I)
    gate, up = split(inter, 2, axis=-1)
    act = silu(gate) * up                          # (C,I)
    out[e] = act @ down_w[e]                       # (C,I)@(I,H) -> (C,H)

E == n_cores == 8, so each core owns one expert end-to-end (no collectives).

Device layout: everything transposed so the PE's contraction dim sits on
partitions for both operands (lhsT = weights stationary, rhs = Xt moving):

    interT[f, c] = sum_h Wgu[h, f] * Xt[h, c]
    outT[h, c]   = sum_i Wd[i, h] * actT[i, c]

The classical kernel streams 4224 N=512 matmuls at the PE's 1-column/cycle
floor (96.5% MFU) -- the only way faster is fewer matmuls.  mm1 (2/3 of the
MACs) uses one level of Strassen over (F2, H, C); the F2 split lands exactly
on the gate|up boundary:

    M1 = (A11+A22)(B11+B22)   M2 = (A21+A22)B11    M3 = A11(B12-B22)
    M4 = A22(B21-B11)         M5 = (A11+A12)B22    M6 = (A21-A11)(B11+B12)
    M7 = (A12-A22)(B21+B22)

    gateT[:, c<1024] = M1+M4-M5+M7      gateT[:, c>=1024] = M3+M5
    upT[:, c<1024]   = M2+M4            upT[:, c>=1024]   = M1-M2+M3+M6

7/8 of the matmuls: 22 f-blocks x 7 products x (16 k-tile MMs across two
PSUM banks).  Both operand combos are PRECOMPUTED ON HOST and DMA'd in
(they are pure functions of the inputs), so the device does only matmuls,
PSUM drains (fused +/- on the Vector engine), Silu-GLU, and DMA.  act
spills to DRAM scratch (SBUF holds the 7 Xt-combos instead) and reloads in
512-column chunks for a c-outer classical mm2.
"""

from contextlib import ExitStack

import numpy as np
import ml_dtypes

E, C, H, I = 8, 2048, 2048, 2816
F2 = 2 * I          # fused gate+up columns
P = 128             # partitions
NF = 512            # PSUM bank of fp32; mm chains write [P, 2*NF] tiles
KT2 = (H // 2) // P  # 8 k-tiles per H-half (Strassen mm1)
FB = I // P         # 22 f-blocks (gate tile i pairs with up tile i)
IT = I // P         # 22 i-tiles over I (mm2 contraction)
HT = H // P         # 16 h-tiles over H (mm2 output)
CT = C // NF        # 4 c-chunks of 512
CH = C // 2         # 1024, Strassen c-half
IT2 = (I // 2) // P  # 11 k-tiles per I-half (Strassen mm2)

_NC_CACHE = {}


def _build_nc(compute="float16"):
    if compute in _NC_CACHE:
        return _NC_CACHE[compute]

    import concourse.bacc as bacc
    import concourse.tile as tile
    from concourse import mybir

    cdt = getattr(mybir.dt, compute)
    f32 = mybir.dt.float32
    AFT = mybir.ActivationFunctionType
    ALU = mybir.AluOpType

    nc = bacc.Bacc(None, target_bir_lowering=False, name="moe_expert_ffn")

    # xt raw (combos formed on DVE: startup is DMA-bound, so ship the
    # smaller raw tensor); weight combos precomputed on host.
    xt_d = nc.dram_tensor("xt", [H, C], cdt, kind="ExternalInput")
    wc_d = nc.dram_tensor("wc", [FB, P, 7, KT2, P], cdt, kind="ExternalInput")
    wd_d = nc.dram_tensor("wd", [HT // 2, P, 7, IT2, P], cdt, kind="ExternalInput")
    odt = cdt if compute == "float16" else f32
    outT_d = nc.dram_tensor("outT", [H, C], odt, kind="ExternalOutput")

    xt_r = xt_d.ap().rearrange("(kt p) c -> p kt c", p=P)       # [128, 16, C]
    wc_a = wc_d.ap()
    wd_a = wd_d.ap()
    outT_a = outT_d.ap()

    with tile.TileContext(nc) as tc, ExitStack() as ctx:
        singles = ctx.enter_context(tc.tile_pool(name="singles", bufs=1))
        wpool = ctx.enter_context(tc.tile_pool(name="wpool", bufs=2))
        accp = ctx.enter_context(tc.tile_pool(name="accp", bufs=2))
        spool = ctx.enter_context(tc.tile_pool(name="spool", bufs=2))
        rpool = ctx.enter_context(tc.tile_pool(name="rpool", bufs=2))
        opool = ctx.enter_context(tc.tile_pool(name="opool", bufs=3))
        psum = ctx.enter_context(tc.tile_pool(name="psum", bufs=2, space="PSUM"))
        dram = ctx.enter_context(tc.tile_pool(name="dram", bufs=1, space="DRAM"))

        # DRAM scratch for the act spill (actT, one tile per i-block).
        act_d = dram.tile([FB, P, C], cdt, tag="actd", name="act_spill")

        # Dummy matmuls on zeroed tiles fill the PE's dead window while the
        # first inputs stream in (HAM un-throttles after ~3.4us of activity).
        wz = singles.tile([P, P], cdt, tag="wz", name="wz")
        xz = singles.tile([P, NF], cdt, tag="xz", name="xz")
        nc.vector.memset(wz, 0.0)
        nc.vector.memset(xz, 0.0)
        warm_ps = psum.tile([P, 2 * NF], f32, tag="pa", name="warm_ps")
        for w in range(24):
            nc.tensor.matmul(
                warm_ps[:, :NF], wz, xz, start=True, stop=True
            )

        # first two weight-combo blocks: highest DMA priority so f-blocks
        # 0/1 never wait on weights behind the xt stream
        wc0 = wpool.tile([P, 7, KT2, P], cdt, tag="wc", name="wc0")
        nc.sync.dma_start(out=wc0, in_=wc_a[0])
        wc1 = wpool.tile([P, 7, KT2, P], cdt, tag="wc", name="wc1")
        nc.sync.dma_start(out=wc1, in_=wc_a[1])

        # Xt combos formed per k-tile on the Vector engine as the raw
        # quadrant slices stream in; one tile per (product, kt) slice so
        # DMAs and readers don't false-share dependency state.
        bc = [[None] * KT2 for _ in range(7)]
        for m in range(7):
            for kt in range(KT2):
                bc[m][kt] = singles.tile(
                    [P, CH], cdt, tag=f"b{m}_{kt}", name=f"b{m}_{kt}"
                )
        for kt in range(KT2):
            nc.sync.dma_start(out=bc[1][kt], in_=xt_r[:, kt, 0:CH])        # B11
            nc.sync.dma_start(out=bc[4][kt], in_=xt_r[:, KT2 + kt, CH:C])  # B22
            t12 = spool.tile([P, CH], cdt, tag="q12", name=f"q12_{kt}")
            t21 = spool.tile([P, CH], cdt, tag="q21", name=f"q21_{kt}")
            nc.sync.dma_start(out=t12, in_=xt_r[:, kt, CH:C])              # B12
            nc.sync.dma_start(out=t21, in_=xt_r[:, KT2 + kt, 0:CH])        # B21
            nc.vector.tensor_add(bc[0][kt], bc[1][kt], bc[4][kt])  # B11+B22
            nc.vector.tensor_sub(bc[2][kt], t12, bc[4][kt])        # B12-B22
            nc.vector.tensor_sub(bc[3][kt], t21, bc[1][kt])        # B21-B11
            nc.vector.tensor_add(bc[5][kt], bc[1][kt], t12)        # B11+B12
            nc.vector.tensor_add(bc[6][kt], t21, bc[4][kt])        # B21+B22

        # per-product drain plan: (acc key, how); "sub" is acc - M fused
        plan = [
            [("gl", "copy"), ("uh", "copy")],          # M1
            [("ul", "copy"), ("uh", "sub")],           # M2
            [("gh", "copy"), ("uh", "add")],           # M3
            [("gl", "add"), ("ul", "add")],            # M4
            [("gh", "add"), ("gl", "sub")],            # M5
            [("uh", "add")],                           # M6
            [("gl", "add")],                           # M7
        ]

        # ---- mm1: per f-block, 7 Strassen products + drains + GLU ----
        for i in range(FB):
            if i == 0:
                wc_t = wc0
            elif i == 1:
                wc_t = wc1
            else:
                wc_t = wpool.tile([P, 7, KT2, P], cdt, tag="wc", name=f"wc{i}")
                nc.sync.dma_start(out=wc_t, in_=wc_a[i])

            acc = {}
            for nm in ("gl", "gh", "ul", "uh"):
                acc[nm] = accp.tile([P, 2 * NF], cdt, tag=nm, name=f"{nm}_{i}")

            def drain(m, ps):
                for key, how in plan[m]:
                    a = acc[key]
                    if how == "copy":
                        nc.vector.tensor_copy(out=a, in_=ps)
                    elif how == "add":
                        nc.vector.tensor_add(a, ps, a)
                    else:  # a = (ps * -1) + a
                        nc.vector.scalar_tensor_tensor(
                            out=a, in0=ps, scalar=-1.0, in1=a,
                            op0=ALU.mult, op1=ALU.add,
                        )

            def mm(ps, m, kt, j):
                nc.tensor.matmul(
                    ps[:, j * NF : (j + 1) * NF],
                    wc_t[:, m, kt, :],
                    bc[m][kt][:, j * NF : (j + 1) * NF],
                    start=(kt == 0),
                    stop=(kt == KT2 - 1),
                )

            if i == 0:
                # kt-major waves: the PE starts the moment the first k-tile
                # combos exist instead of waiting for full products' worth
                # of Xt (startup is DMA-bound).
                pss = {}
                for wave in ((0, 1, 2, 3), (4, 5, 6)):
                    for m in wave:
                        pss[m] = psum.tile(
                            [P, 2 * NF], f32, tag="pa" if m % 2 == 0 else "pb",
                            name=f"m{m}_{i}",
                        )
                    for kt in range(KT2):
                        for m in wave:
                            for j in range(2):
                                mm(pss[m], m, kt, j)
                for m in range(7):
                    drain(m, pss[m])
            else:
                for m in range(7):
                    ps = psum.tile(
                        [P, 2 * NF], f32, tag="pa" if m % 2 == 0 else "pb",
                        name=f"m{m}_{i}",
                    )
                    for j in range(2):
                        for kt in range(KT2):
                            mm(ps, m, kt, j)
                    drain(m, ps)

            act_t = spool.tile([P, C], cdt, tag="act", name=f"act{i}")
            for half, g, u in ((0, "gl", "ul"), (1, "gh", "uh")):
                s_sb = spool.tile([P, 2 * NF], cdt, tag=f"sig{half}", name=f"sig{half}_{i}")
                nc.scalar.activation(out=s_sb, in_=acc[g], func=AFT.Silu)
                nc.vector.tensor_mul(
                    act_t[:, half * CH : (half + 1) * CH], s_sb, acc[u]
                )
            nc.sync.dma_start(out=act_d[i], in_=act_t)

        # ---- mm2: Strassen over (H, I, Cg), two c-groups of 1024 ----
        # act combos [I/2, 512] stream from the spill as (kt-pair) tiles
        # that reuse the Xt-combo tag buffers; the tag-free cascade off
        # mm1's last f-block (and off the previous group's last h-pair)
        # overlaps formation with compute.  Wd combos are host-packed.
        for g in range(2):
            g0 = g * CH
            acomb = [[None] * 6 for _ in range(7)]
            for m in range(7):
                for t in range(6):
                    ti = m * 6 + t
                    acomb[m][t] = singles.tile(
                        [P, 2, NF], cdt, tag=f"b{ti // KT2}_{ti % KT2}",
                        name=f"ac{m}_{t}_{g}",
                    )

            def acf(m, kt):
                return acomb[m][kt // 2][:, kt % 2, :]

            for kt in range(IT2):
                nc.sync.dma_start(
                    out=acf(1, kt), in_=act_d[kt, :, g0 : g0 + NF]
                )
                nc.sync.dma_start(
                    out=acf(4, kt), in_=act_d[IT2 + kt, :, g0 + NF : g0 + CH]
                )
                t12 = spool.tile([P, NF], cdt, tag="q12", name=f"a12_{kt}_{g}")
                t21 = spool.tile([P, NF], cdt, tag="q21", name=f"a21_{kt}_{g}")
                nc.sync.dma_start(out=t12, in_=act_d[kt, :, g0 + NF : g0 + CH])
                nc.sync.dma_start(out=t21, in_=act_d[IT2 + kt, :, g0 : g0 + NF])
                nc.vector.tensor_add(acf(0, kt), acf(1, kt), acf(4, kt))
                nc.vector.tensor_sub(acf(2, kt), t12, acf(4, kt))
                nc.vector.tensor_sub(acf(3, kt), t21, acf(1, kt))
                nc.vector.tensor_add(acf(5, kt), acf(1, kt), t12)
                nc.vector.tensor_add(acf(6, kt), t21, acf(4, kt))

            for j in range(HT // 2):
                wds_t = wpool.tile(
                    [P, 7, IT2, P], cdt, tag="wc", name=f"wds{j}_{g}"
                )
                nc.sync.dma_start(out=wds_t, in_=wd_a[j])
                oacc = {
                    nm: accp.tile([P, NF], odt, tag=nm, name=f"o{nm}{j}_{g}")
                    for nm in ("gl", "gh", "ul", "uh")
                }
                for m in range(7):
                    ps = psum.tile(
                        [P, 2 * NF], f32, tag="pa" if m % 2 == 0 else "pb",
                        name=f"o{m}_{j}_{g}",
                    )
                    for kt in range(IT2):
                        nc.tensor.matmul(
                            ps[:, :NF],
                            wds_t[:, m, kt, :],
                            acf(m, kt),
                            start=(kt == 0),
                            stop=(kt == IT2 - 1),
                        )
                    for key, how in plan[m]:
                        a = oacc[key]
                        if how == "copy":
                            nc.vector.tensor_copy(out=a, in_=ps[:, :NF])
                        elif how == "add":
                            nc.vector.tensor_add(a, ps[:, :NF], a)
                        else:
                            nc.vector.scalar_tensor_tensor(
                                out=a, in0=ps[:, :NF], scalar=-1.0, in1=a,
                                op0=ALU.mult, op1=ALU.add,
                            )
                for nm, hrow, chalf in (
                    ("gl", j, 0), ("gh", j, 1),
                    ("ul", j + HT // 2, 0), ("uh", j + HT // 2, 1),
                ):
                    h0 = hrow * P
                    c0 = g0 + chalf * NF
                    nc.sync.dma_start(
                        out=outT_a[h0 : h0 + P, c0 : c0 + NF], in_=oacc[nm]
                    )

    nc.compile()
    _NC_CACHE[compute] = nc
    return nc


def _np_dtype(compute):
    return {"bfloat16": ml_dtypes.bfloat16, "float16": np.float16, "float32r": np.float32}[compute]


def _pack_wds(wd):
    """[I, H] -> Strassen stationary combos [HT//2, P, 7, IT2, P]."""
    out = np.empty((HT // 2, P, 7, IT2, P), dtype=wd.dtype)
    I2 = I // 2
    for j in range(HT // 2):
        hj = wd[:, j * P : (j + 1) * P]                      # h-low cols
        hj8 = wd[:, H // 2 + j * P : H // 2 + (j + 1) * P]   # h-high cols
        q0, q1 = hj[:I2], hj[I2:]
        q2, q3 = hj8[:I2], hj8[I2:]
        combos = (q0 + q3, q2 + q3, q0, q3, q0 + q1, q2 - q0, q1 - q3)
        for m, cb in enumerate(combos):
            # [1408, 128] -> [kt, p, f] -> [p, kt, f]
            out[j, :, m] = cb.reshape(IT2, P, P).transpose(1, 0, 2)
    return out


def _pack_wc(wgu):
    """[H, F2] -> Strassen stationary combos [FB, P, 7, KT2, P]."""
    out = np.empty((FB, P, 7, KT2, P), dtype=wgu.dtype)
    for i in range(FB):
        g = wgu[:, i * P : (i + 1) * P]            # [H, 128] gate cols
        u = wgu[:, I + i * P : I + (i + 1) * P]    # [H, 128] up cols
        g_lo, g_hi = g[: H // 2], g[H // 2 :]
        u_lo, u_hi = u[: H // 2], u[H // 2 :]
        combos = (g_lo + u_hi, u_lo + u_hi, g_lo, u_hi, g_lo + g_hi,
                  u_lo - g_lo, g_hi - u_hi)
        for m, cb in enumerate(combos):
            # [1024, 128] -> [kt, p, f] -> [p, kt, f]
            out[i, :, m] = cb.reshape(KT2, P, P).transpose(1, 0, 2)
    return out


def make_in_maps(hidden_states, gate_up_w, down_w, compute="float16"):
    dt = _np_dtype(compute)
    in_maps = []
    for e in range(E):
        in_maps.append(
            {
                "xt": np.ascontiguousarray(hidden_states[e].T).astype(dt),
                "wc": _pack_wc(gate_up_w[e].astype(dt)),
                "wd": _pack_wds(down_w[e].astype(dt)),
            }
        )
    return in_maps


def run_hw(in_maps, compute="float16", trace=False, **kwargs):
    from concourse import bass_utils

    if trace:
        # local-only devloop: skip the artifact-bucket upload
        bass_utils.upload_artifacts = lambda tmpdir: f"local:{tmpdir}"
    nc = _build_nc(compute)
    return bass_utils.run_bass_kernel_spmd(
        nc, in_maps, core_ids=list(range(E)), trace=trace, **kwargs
    )


def kernel(hidden_states, gate_up_w, down_w):
    compute = "float16"
    hidden_states = np.asarray(hidden_states)
    gate_up_w = np.asarray(gate_up_w)
    down_w = np.asarray(down_w)
    in_maps = make_in_maps(hidden_states, gate_up_w, down_w, compute)
    res = run_hw(in_maps, compute)
    out = np.empty((E, C, H), dtype=np.float32)
    for e in range(E):
        out[e] = res.results[e]["outT"].T
    return out
